# revision 3
# baseline (speedup 1.0000x reference)
"""BinaryDiff kernel for 8 TRN2 NeuronCores.

Computes out = x @ base + coeff * (x @ (2*mask - 1)) for
x [4,2048,4096] f32, base [4096,4096] f32, mask [4096,4096] i32,
coeff [] f32 -> out [4,2048,4096] f32.

Key algebraic fusion: dense + coeff*binary = x @ (base + coeff*(2*mask-1)),
so we fuse the weights on-device (one elementwise pass) and run a SINGLE
matmul in bf16 (fp32 PSUM accumulation).

Sharding (tensor-parallel 2x4 grid, no collectives):
  - rows (B*S = 8192) split 2 ways  -> 4096 rows/core
  - out cols (4096)   split 4 ways  -> 1024 cols/core

Host-side marshaling (part of the sharding step): x is pre-tiled into the
k-major layout the PE's stationary operand wants —
  xh[b*128 + p, kt*128 + j] = x[b*128 + j, kt*128 + p]   (bf16)
— so each 128-row block's whole K panel is ONE contiguous 1 MB DMA and
matmul lhsT tiles are direct SBUF slices: no PE transposes, no on-chip
casts.  base and sign = (2*mask-1) are sent as bf16 (per-element dtype
maps); the W = base + coeff*sign fusion itself runs on device (gpsimd).

Device schedule per core:
  Phase A (blocks 0..7): K split in rounds [4,4,8,8,8]; round partials
    land in PSUM and are combined into an SBUF accumulator (ScalarE copy
    for round 0, DVE adds after), while the NEXT round's W-fusion DMAs
    and x chunks stream in.  This hides the 16.8 MB of W-source DMA
    behind the first ~110 us of matmuls.
  Phase B (blocks 8..31): all W resident; each block is 64 back-to-back
    512-cycle matmuls accumulating full K=4096 in PSUM (2 banks), then a
    ScalarE evacuation to SBUF and a store on the ACT DMA ring (separate
    from the load ring).
PE therefore does nothing but 2048 N=512 bf16 matmuls: ~437 us roofline.
"""

import numpy as np
from contextlib import ExitStack

import ml_dtypes

import concourse.bass as bass
import concourse.mybir as mybir
import concourse.tile as tile
from concourse import bacc
from concourse.bass_utils import run_bass_kernel_spmd

P = 128
B, S, DIN, DOUT = 4, 2048, 4096, 4096
P_ROWS, Q_COLS = 2, 4           # core grid: 2 row-shards x 4 col-shards
BS = B * S                      # 8192
BS_C = BS // P_ROWS             # 4096 rows per core
NO_C = DOUT // Q_COLS           # 1024 out cols per core
MM_N = 512                      # matmul moving free dim (1 PSUM bank of f32)

f32 = mybir.dt.float32
bf16 = mybir.dt.bfloat16
bf16_np = ml_dtypes.bfloat16


def emit_kernel(tc, xh_ap, base_ap, sign_ap, coeff_ap, out_ap,
                bs_c, din, no_c):
    """Emit the per-core Tile program. Shapes parameterized for sim tests."""
    nc = tc.nc
    kt_n = din // P            # k tiles
    nblk = bs_c // P           # 128-row x blocks
    ot_n = max(1, no_c // MM_N)
    mm_n = min(MM_N, no_c)
    ga = min(8, nblk)          # phase-A block count

    # K-round ramp for phase A: small first rounds so matmuls start after
    # a short fusion prologue, 8-tile rounds at steady state.
    rounds = []
    rem, lo = kt_n, 0
    for sz in [4, 4] + [8] * ((kt_n + 7) // 8):
        if rem == 0:
            break
        s = min(sz, rem)
        rounds.append((lo, lo + s))
        lo += s
        rem -= s
    max_kq = max(hi - lo for lo, hi in rounds)

    with ExitStack() as ctx:
        const = ctx.enter_context(tc.tile_pool(name="const", bufs=1))
        wpool = ctx.enter_context(tc.tile_pool(name="wpool", bufs=kt_n))
        fb = ctx.enter_context(tc.tile_pool(name="fbase", bufs=3))
        fs = ctx.enter_context(tc.tile_pool(name="fsgn", bufs=3))
        xap = ctx.enter_context(tc.tile_pool(name="xa", bufs=2 * ga + 1))
        xbp = ctx.enter_context(tc.tile_pool(name="xb", bufs=3))
        evp = ctx.enter_context(tc.tile_pool(name="ev", bufs=ga + 1))
        mmp = ctx.enter_context(tc.tile_pool(name="mmpsum", bufs=8, space="PSUM"))

        # --- coeff broadcast: [128,1] = ones.T @ coeff ---
        c_sb = const.tile([1, 1], f32)
        nc.sync.dma_start(c_sb[:], coeff_ap[:])
        ones = const.tile([1, P], f32)
        nc.any.memset(ones[:], 1.0)
        cps = mmp.tile([P, mm_n], f32, tag="ps")
        nc.tensor.matmul(cps[:, 0:1], ones[:], c_sb[:], start=True, stop=True)
        c_t = const.tile([P, 1], f32)
        nc.vector.tensor_copy(c_t[:], cps[:, 0:1])

        # --- W fusion: W[kt] = bf16(base + c*sign), SBUF resident ---
        wtiles = [None] * kt_n

        def emit_fusion(kt):
            bt = fb.tile([P, no_c], bf16, tag="fb", name="bt")
            nc.sync.dma_start(bt[:], base_ap[kt * P:(kt + 1) * P, :])
            sg = fs.tile([P, no_c], bf16, tag="fs", name="sg")
            nc.sync.dma_start(sg[:], sign_ap[kt * P:(kt + 1) * P, :])
            wt = wpool.tile([P, no_c], bf16, tag="w", name="wt")
            nc.gpsimd.scalar_tensor_tensor(
                wt[:], sg[:], c_t[:], bt[:],
                mybir.AluOpType.mult, mybir.AluOpType.add)
            wtiles[kt] = wt

        # --- phase A ---
        ev_of = {}
        xa_of = {}

        def emit_chunks(si):
            """x chunk DMAs for stage si (all ga blocks), fusion for round si
            woven 1:1 across the blocks."""
            klo, khi = rounds[si]
            kts = list(range(klo, khi))
            for i, b in enumerate(range(ga)):
                xa = xap.tile([P, max_kq * P], bf16, tag="xa", name="xa")
                nc.sync.dma_start(
                    xa[:, 0:(khi - klo) * P],
                    xh_ap[b * P:(b + 1) * P, klo * P:khi * P])
                xa_of[(si, b)] = xa
                for kt in kts[len(kts) * i // ga:len(kts) * (i + 1) // ga]:
                    emit_fusion(kt)

        emit_fusion_done = [False]

        def emit_round(b, si, first, last):
            klo, khi = rounds[si]
            xa = xa_of.pop((si, b))
            if first:
                ev_of[b] = evp.tile([P, no_c], f32, tag="ev", name="ev")
            ev = ev_of[b]
            for ot in range(ot_n):
                ps = mmp.tile([P, mm_n], f32, tag="ps", name="ps")
                for kt in range(klo, khi):
                    nc.tensor.matmul(
                        ps[:],
                        xa[:, (kt - klo) * P:(kt - klo + 1) * P],
                        wtiles[kt][:, ot * mm_n:(ot + 1) * mm_n],
                        start=(kt == klo), stop=(kt == khi - 1),
                    )
                evs = ev[:, ot * mm_n:(ot + 1) * mm_n]
                if first:
                    nc.scalar.copy(evs, ps[:])
                else:
                    nc.vector.tensor_tensor(evs, evs, ps[:],
                                            mybir.AluOpType.add)
            if last:
                nc.scalar.dma_start(out_ap[b * P:(b + 1) * P, :], ev[:])

        # prologue: round-0 fusion + stage-0 x chunks
        emit_chunks(0)
        for si in range(len(rounds)):
            if si + 1 < len(rounds):
                emit_chunks(si + 1)
            else:
                # last phase-A stage: start prefetching phase B
                for b in range(ga, min(ga + 2, nblk)):
                    xb = xbp.tile([P, din], bf16, tag="xb", name="xb")
                    nc.sync.dma_start(xb[:], xh_ap[b * P:(b + 1) * P, :])
                    xa_of[("B", b)] = xb
            for b in range(ga):
                emit_round(b, si, first=(si == 0), last=(si == len(rounds) - 1))
        for b in range(ga):
            del ev_of[b]

        # --- phase B: full-K PSUM accumulation per block ---
        for b in range(ga, nblk):
            nxt = b + 2
            if nxt < nblk:
                xb = xbp.tile([P, din], bf16, tag="xb", name="xb")
                nc.sync.dma_start(xb[:], xh_ap[nxt * P:(nxt + 1) * P, :])
                xa_of[("B", nxt)] = xb
            xb = xa_of.pop(("B", b))
            pss = [mmp.tile([P, mm_n], f32, tag="ps", name="ps")
                   for _ in range(ot_n)]
            for kt in range(kt_n):
                for ot in range(ot_n):
                    nc.tensor.matmul(
                        pss[ot][:],
                        xb[:, kt * P:(kt + 1) * P],
                        wtiles[kt][:, ot * mm_n:(ot + 1) * mm_n],
                        start=(kt == 0), stop=(kt == kt_n - 1),
                    )
            ev = evp.tile([P, no_c], f32, tag="ev", name="ev")
            for ot in range(ot_n):
                nc.scalar.copy(ev[:, ot * mm_n:(ot + 1) * mm_n], pss[ot][:])
            nc.scalar.dma_start(out_ap[b * P:(b + 1) * P, :], ev[:])


def build_nc(bs_c=BS_C, din=DIN, no_c=NO_C):
    nc = bacc.Bacc("TRN2", target_bir_lowering=False, debug=False, num_devices=8)
    xh_ap = nc.dram_tensor("xh", [bs_c, din], bf16, kind="ExternalInput").ap()
    base_ap = nc.dram_tensor("base", [din, no_c], bf16, kind="ExternalInput").ap()
    sign_ap = nc.dram_tensor("sign", [din, no_c], bf16, kind="ExternalInput").ap()
    coeff_ap = nc.dram_tensor("coeff", [1, 1], f32, kind="ExternalInput").ap()
    out_ap = nc.dram_tensor("out", [bs_c, no_c], f32, kind="ExternalOutput").ap()
    with tile.TileContext(nc) as tc:
        emit_kernel(tc, xh_ap, base_ap, sign_ap, coeff_ap, out_ap,
                    bs_c, din, no_c)
    nc.compile()
    return nc


_NC_CACHE = {}


def _get_nc():
    if "nc" not in _NC_CACHE:
        _NC_CACHE["nc"] = build_nc()
    return _NC_CACHE["nc"]


def prep_x_shard(xs):
    """[rows, din] f32 -> k-major tiled bf16: out[b*P+p, t*P+j] = xs[b*P+j, t*P+p]."""
    nb, kt = xs.shape[0] // P, xs.shape[1] // P
    t = xs.astype(bf16_np).reshape(nb, P, kt, P).transpose(0, 3, 2, 1)
    return np.ascontiguousarray(t.reshape(nb * P, kt * P))


def make_in_maps(x, base, mask, coeff):
    """Shard full inputs across the 2x4 core grid (cores 0..7)."""
    xf = np.ascontiguousarray(x.reshape(BS, DIN).astype(np.float32, copy=False))
    xh = [prep_x_shard(xf[pi * BS_C:(pi + 1) * BS_C]) for pi in range(P_ROWS)]
    coeff2d = np.asarray(coeff, dtype=np.float32).reshape(1, 1)
    sign = (2 * mask.astype(np.int32) - 1)
    in_maps = []
    for cid in range(8):
        pi, qi = divmod(cid, Q_COLS)
        cs = slice(qi * NO_C, (qi + 1) * NO_C)
        in_maps.append({
            "xh": xh[pi],
            "base": np.ascontiguousarray(base[:, cs]).astype(bf16_np),
            "sign": np.ascontiguousarray(sign[:, cs]).astype(bf16_np),
            "coeff": coeff2d,
        })
    return in_maps


def assemble_out(results):
    out = np.empty((BS, DOUT), dtype=np.float32)
    for cid in range(8):
        pi, qi = divmod(cid, Q_COLS)
        out[pi * BS_C:(pi + 1) * BS_C, qi * NO_C:(qi + 1) * NO_C] = \
            results[cid]["out"]
    return out.reshape(B, S, DOUT)


def kernel(x, base, mask, coeff):
    nc = _get_nc()
    in_maps = make_in_maps(np.asarray(x), np.asarray(base),
                           np.asarray(mask), np.asarray(coeff))
    res = run_bass_kernel_spmd(nc, in_maps, core_ids=list(range(8)))
    return assemble_out(res.results)


# revision 4
# speedup vs baseline: 1.2393x; 1.2393x over previous
"""BinaryDiff kernel for 8 TRN2 NeuronCores.

Computes out = x @ base + coeff * (x @ (2*mask - 1)) for
x [4,2048,4096] f32, base [4096,4096] f32, mask [4096,4096] i32,
coeff [] f32 -> out [4,2048,4096] f32.

Key algebraic fusion: dense + coeff*binary = x @ (base + coeff*(2*mask-1)),
so we fuse the weights on-device (one elementwise pass) and run a SINGLE
matmul in bf16 (fp32 PSUM accumulation).

Sharding (tensor-parallel 2x4 grid, no collectives):
  - rows (B*S = 8192) split 2 ways  -> 4096 rows/core
  - out cols (4096)   split 4 ways  -> 1024 cols/core

Host-side marshaling (part of the sharding step): x is pre-tiled into the
k-major layout the PE's stationary operand wants —
  xh[b*128 + p, kt*128 + j] = x[b*128 + j, kt*128 + p]   (bf16)
— so each 128-row block's whole K panel is ONE contiguous 1 MB DMA and
matmul lhsT tiles are direct SBUF slices: no PE transposes, no on-chip
casts.  base and sign = (2*mask-1) are sent as bf16 (per-element dtype
maps); the W = base + coeff*sign fusion itself runs on device (gpsimd).

Device schedule per core:
  Phase A (blocks 0..7): K split in rounds [4,4,8,8,8]; round partials
    land in PSUM and are combined into an SBUF accumulator (ScalarE copy
    for round 0, DVE adds after), while the NEXT round's W-fusion DMAs
    and x chunks stream in.  This hides the 16.8 MB of W-source DMA
    behind the first ~110 us of matmuls.
  Phase B (blocks 8..31): all W resident; each block is 64 back-to-back
    512-cycle matmuls accumulating full K=4096 in PSUM (2 banks), then a
    ScalarE evacuation to SBUF and a store on the ACT DMA ring (separate
    from the load ring).
PE therefore does nothing but 2048 N=512 bf16 matmuls: ~437 us roofline.
"""

import numpy as np
from contextlib import ExitStack

import ml_dtypes

import concourse.bass as bass
import concourse.mybir as mybir
import concourse.tile as tile
from concourse import bacc
from concourse.bass_utils import run_bass_kernel_spmd

P = 128
B, S, DIN, DOUT = 4, 2048, 4096, 4096
P_ROWS, Q_COLS = 2, 4           # core grid: 2 row-shards x 4 col-shards
BS = B * S                      # 8192
BS_C = BS // P_ROWS             # 4096 rows per core
NO_C = DOUT // Q_COLS           # 1024 out cols per core
MM_N = 512                      # matmul moving free dim (1 PSUM bank of f32)

f32 = mybir.dt.float32
bf16 = mybir.dt.bfloat16
bf16_np = ml_dtypes.bfloat16


def emit_kernel(tc, xh_ap, base_ap, sign_ap, coeff_ap, out_ap,
                bs_c, din, no_c):
    """Emit the per-core Tile program. Shapes parameterized for sim tests."""
    nc = tc.nc
    kt_n = din // P            # k tiles
    nblk = bs_c // P           # 128-row x blocks
    ot_n = max(1, no_c // MM_N)
    mm_n = min(MM_N, no_c)
    ga = min(8, nblk)          # phase-A block count

    # K-round ramp for phase A: small first rounds so matmuls start after
    # a short fusion prologue, 8-tile rounds at steady state.
    rounds = []
    rem, lo = kt_n, 0
    for sz in [4, 4] + [8] * ((kt_n + 7) // 8):
        if rem == 0:
            break
        s = min(sz, rem)
        rounds.append((lo, lo + s))
        lo += s
        rem -= s
    max_kq = max(hi - lo for lo, hi in rounds)

    with ExitStack() as ctx:
        const = ctx.enter_context(tc.tile_pool(name="const", bufs=1))
        wpool = ctx.enter_context(tc.tile_pool(name="wpool", bufs=kt_n))
        fb = ctx.enter_context(tc.tile_pool(name="fbase", bufs=3))
        fs = ctx.enter_context(tc.tile_pool(name="fsgn", bufs=3))
        xap = ctx.enter_context(tc.tile_pool(name="xa", bufs=2 * ga + 1))
        xbp = ctx.enter_context(tc.tile_pool(name="xb", bufs=3))
        evp = ctx.enter_context(tc.tile_pool(name="ev", bufs=ga + 1))
        mmp = ctx.enter_context(tc.tile_pool(name="mmpsum", bufs=8, space="PSUM"))

        # --- coeff broadcast: [128,1] = ones.T @ coeff ---
        c_sb = const.tile([1, 1], f32)
        nc.sync.dma_start(c_sb[:], coeff_ap[:])
        ones = const.tile([1, P], f32)
        nc.any.memset(ones[:], 1.0)
        cps = mmp.tile([P, mm_n], f32, tag="ps")
        nc.tensor.matmul(cps[:, 0:1], ones[:], c_sb[:], start=True, stop=True)
        c_t = const.tile([P, 1], f32)
        nc.vector.tensor_copy(c_t[:], cps[:, 0:1])

        # --- W fusion: W[kt] = bf16(base + c*sign), SBUF resident ---
        wtiles = [None] * kt_n

        def emit_fusion(kt):
            bt = fb.tile([P, no_c], bf16, tag="fb", name="bt")
            nc.sync.dma_start(bt[:], base_ap[kt * P:(kt + 1) * P, :])
            sg = fs.tile([P, no_c], bf16, tag="fs", name="sg")
            nc.sync.dma_start(sg[:], sign_ap[kt * P:(kt + 1) * P, :])
            wt = wpool.tile([P, no_c], bf16, tag="w", name="wt")
            nc.vector.scalar_tensor_tensor(
                wt[:], sg[:], c_t[:], bt[:],
                mybir.AluOpType.mult, mybir.AluOpType.add)
            wtiles[kt] = wt

        # --- phase A ---
        ev_of = {}
        xa_of = {}

        def emit_chunks(si):
            """x chunk DMAs for stage si (all ga blocks), fusion for round si
            woven 1:1 across the blocks."""
            klo, khi = rounds[si]
            kts = list(range(klo, khi))
            for i, b in enumerate(range(ga)):
                xa = xap.tile([P, max_kq * P], bf16, tag="xa", name="xa")
                nc.sync.dma_start(
                    xa[:, 0:(khi - klo) * P],
                    xh_ap[b * P:(b + 1) * P, klo * P:khi * P])
                xa_of[(si, b)] = xa
                for kt in kts[len(kts) * i // ga:len(kts) * (i + 1) // ga]:
                    emit_fusion(kt)

        emit_fusion_done = [False]

        def emit_round(b, si, first, last):
            klo, khi = rounds[si]
            xa = xa_of.pop((si, b))
            if first:
                ev_of[b] = evp.tile([P, no_c], f32, tag="ev", name="ev")
            ev = ev_of[b]
            for ot in range(ot_n):
                ps = mmp.tile([P, mm_n], f32, tag="ps", name="ps")
                for kt in range(klo, khi):
                    nc.tensor.matmul(
                        ps[:],
                        xa[:, (kt - klo) * P:(kt - klo + 1) * P],
                        wtiles[kt][:, ot * mm_n:(ot + 1) * mm_n],
                        start=(kt == klo), stop=(kt == khi - 1),
                    )
                evs = ev[:, ot * mm_n:(ot + 1) * mm_n]
                if first:
                    nc.scalar.copy(evs, ps[:])
                else:
                    nc.vector.tensor_tensor(evs, evs, ps[:],
                                            mybir.AluOpType.add)
            if last:
                nc.scalar.dma_start(out_ap[b * P:(b + 1) * P, :], ev[:])

        # prologue: round-0 fusion + stage-0 x chunks
        emit_chunks(0)
        for si in range(len(rounds)):
            if si + 1 < len(rounds):
                emit_chunks(si + 1)
            else:
                # last phase-A stage: start prefetching phase B
                for b in range(ga, min(ga + 2, nblk)):
                    xb = xbp.tile([P, din], bf16, tag="xb", name="xb")
                    nc.sync.dma_start(xb[:], xh_ap[b * P:(b + 1) * P, :])
                    xa_of[("B", b)] = xb
            for b in range(ga):
                emit_round(b, si, first=(si == 0), last=(si == len(rounds) - 1))
        for b in range(ga):
            del ev_of[b]

        # --- phase B: full-K PSUM accumulation per block ---
        for b in range(ga, nblk):
            nxt = b + 2
            if nxt < nblk:
                xb = xbp.tile([P, din], bf16, tag="xb", name="xb")
                nc.sync.dma_start(xb[:], xh_ap[nxt * P:(nxt + 1) * P, :])
                xa_of[("B", nxt)] = xb
            xb = xa_of.pop(("B", b))
            pss = [mmp.tile([P, mm_n], f32, tag="ps", name="ps")
                   for _ in range(ot_n)]
            for kt in range(kt_n):
                for ot in range(ot_n):
                    nc.tensor.matmul(
                        pss[ot][:],
                        xb[:, kt * P:(kt + 1) * P],
                        wtiles[kt][:, ot * mm_n:(ot + 1) * mm_n],
                        start=(kt == 0), stop=(kt == kt_n - 1),
                    )
            ev = evp.tile([P, no_c], f32, tag="ev", name="ev")
            for ot in range(ot_n):
                nc.scalar.copy(ev[:, ot * mm_n:(ot + 1) * mm_n], pss[ot][:])
            nc.scalar.dma_start(out_ap[b * P:(b + 1) * P, :], ev[:])


def build_nc(bs_c=BS_C, din=DIN, no_c=NO_C):
    nc = bacc.Bacc("TRN2", target_bir_lowering=False, debug=False, num_devices=8)
    xh_ap = nc.dram_tensor("xh", [bs_c, din], bf16, kind="ExternalInput").ap()
    base_ap = nc.dram_tensor("base", [din, no_c], bf16, kind="ExternalInput").ap()
    sign_ap = nc.dram_tensor("sign", [din, no_c], bf16, kind="ExternalInput").ap()
    coeff_ap = nc.dram_tensor("coeff", [1, 1], f32, kind="ExternalInput").ap()
    out_ap = nc.dram_tensor("out", [bs_c, no_c], f32, kind="ExternalOutput").ap()
    with tile.TileContext(nc) as tc:
        emit_kernel(tc, xh_ap, base_ap, sign_ap, coeff_ap, out_ap,
                    bs_c, din, no_c)
    nc.compile()
    return nc


_NC_CACHE = {}


def _get_nc():
    if "nc" not in _NC_CACHE:
        _NC_CACHE["nc"] = build_nc()
    return _NC_CACHE["nc"]


def prep_x_shard(xs):
    """[rows, din] f32 -> k-major tiled bf16: out[b*P+p, t*P+j] = xs[b*P+j, t*P+p]."""
    nb, kt = xs.shape[0] // P, xs.shape[1] // P
    t = xs.astype(bf16_np).reshape(nb, P, kt, P).transpose(0, 3, 2, 1)
    return np.ascontiguousarray(t.reshape(nb * P, kt * P))


def make_in_maps(x, base, mask, coeff):
    """Shard full inputs across the 2x4 core grid (cores 0..7)."""
    xf = np.ascontiguousarray(x.reshape(BS, DIN).astype(np.float32, copy=False))
    xh = [prep_x_shard(xf[pi * BS_C:(pi + 1) * BS_C]) for pi in range(P_ROWS)]
    coeff2d = np.asarray(coeff, dtype=np.float32).reshape(1, 1)
    sign = (2 * mask.astype(np.int32) - 1)
    in_maps = []
    for cid in range(8):
        pi, qi = divmod(cid, Q_COLS)
        cs = slice(qi * NO_C, (qi + 1) * NO_C)
        in_maps.append({
            "xh": xh[pi],
            "base": np.ascontiguousarray(base[:, cs]).astype(bf16_np),
            "sign": np.ascontiguousarray(sign[:, cs]).astype(bf16_np),
            "coeff": coeff2d,
        })
    return in_maps


def assemble_out(results):
    out = np.empty((BS, DOUT), dtype=np.float32)
    for cid in range(8):
        pi, qi = divmod(cid, Q_COLS)
        out[pi * BS_C:(pi + 1) * BS_C, qi * NO_C:(qi + 1) * NO_C] = \
            results[cid]["out"]
    return out.reshape(B, S, DOUT)


def kernel(x, base, mask, coeff):
    nc = _get_nc()
    in_maps = make_in_maps(np.asarray(x), np.asarray(base),
                           np.asarray(mask), np.asarray(coeff))
    res = run_bass_kernel_spmd(nc, in_maps, core_ids=list(range(8)))
    return assemble_out(res.results)
